# revision 73
# baseline (speedup 1.0000x reference)
"""Multi-head causal attention (B=2,S=2048,D=1024,H=16,dqk=dv=64) on 8 trn2
NeuronCores.

Sharding: tensor-parallel over heads (2 heads/core) for QKV+attention, then an
AllToAll flips to sequence-parallel (512 rows/core) for the output projection.

All matmuls run in bf16: this chip's PE clock governor caps sustained PE
utilization (throttle_activity_1 avg_util_limit = 0.5 -> K=4/8 pulse gating),
so wall time ~ PE cycle count.  x is supplied host-side pre-transposed.

v8 structure (per core, both heads processed together; the PE array is
tiled so concurrent sub-matmuls halve governor-visible busy time):
  A. DMA x^T slices; Q^T/K^T/V^T = W.T @ x^T (feature-on-partition), bias on
     copy-out; V^T -> V via one [128,128] PE transpose per key chunk (both
     heads at once)
  B. flash attention in transposed-score layout, chunk PAIRS:
     - scores: the two K=64 head matmuls are row-tiled onto array halves
       (rows 0-63 / 64-127) sharing the q-column stream -> 512 cycles for
       both heads; both land in one 2-bank psum tile so a single ACT exps
       them at [128,1024] width
     - P@V: col-tiled pair (head0 -> psum partitions 0:64, head1 -> 64:128)
       with M=64 each -> 512 cycles for both heads
     - denominators: per chunk-pair, four M=1 ones-matmuls at col groups
       0..3 (2 heads x 2 chunks) accumulate  sum exp  into rows {0,32,64,96}
       of one psum bank -> 512 cycles per TWO chunks
     - epilogue (no heavy DVE): 1/d = exp(-ln(d_even+d_odd)) on the scalar
       engine, selector matmul broadcasts it, DVE normalizes, slots DMA
       into the per-batch A2A buffer (slot = 256-row quarter tile)
  C. one AllToAll per BATCH: b0's A2A overlaps b1's attention; b0's Wo
     matmuls overlap b1's A2A; only b1's Wo is a serial tail.
Host: interleave the 8 [512,1024] row blocks (256 rows per batch per core).
"""

import numpy as np
import ml_dtypes

import bass_rust
import concourse.bass as bass
import concourse.mybir as mybir
import concourse.tile as tile
from concourse import bass_utils
from concourse.vector_clock import ScopedClock

# ---------------------------------------------------------------------------
# Workaround for this container's walrus build: it accepts at most ONE sync
# wait per instruction, but Tile emits several (tail drain + stage-1B waits).
# Split extra waits onto same-engine NoOps placed right before the instruction.
# ---------------------------------------------------------------------------

_waitsplit_cnt = [0]


def _patched_drain_and_barrier(self, tick_clock, wait_clock):
    nc = self.nc
    drain_inst = nc.sync.drain()
    wait_clock.add_sem_waits(
        drain_inst.ins, ScopedClock({None: tick_clock.global_clock})
    )
    si = drain_inst.ins.sync_info
    waits = list(si.on_wait) if si is not None else []
    if len(waits) > 1:
        drain_inst.ins.sync_info = bass_rust.SyncInfo(
            on_wait=[waits[0]], on_update=list(si.on_update)
        )
        for w in waits[1:]:
            d2 = nc.sync.drain()
            d2.ins.sync_info = bass_rust.SyncInfo(on_wait=[w], on_update=[])
    nc.all_engine_barrier()
    popped = nc._tile_sem_poison_stack.pop()
    assert popped is self._sem_poison
    nc.clear_and_free_semaphores(list(self.sems.allocated().values()))
    nc.all_engine_barrier()


tile.TileContext._drain_and_barrier = _patched_drain_and_barrier


def _split_multi_waits(nc):
    for f in nc.m.functions:
        for bb in f.blocks:
            insts = bb.instructions
            out = []
            dirty = False
            for inst in insts:
                si = inst.sync_info
                if si is not None and len(si.on_wait) > 1:
                    waits = list(si.on_wait)
                    for w in waits[:-1]:
                        nop = mybir.InstNoOp(
                            name=f"waitsplit_{_waitsplit_cnt[0]}", ins=[], outs=[]
                        )
                        _waitsplit_cnt[0] += 1
                        nop.engine = inst.engine
                        nop.sync_info = bass_rust.SyncInfo(on_wait=[w], on_update=[])
                        out.append(nop)
                    inst.sync_info = bass_rust.SyncInfo(
                        on_wait=[waits[-1]], on_update=list(si.on_update)
                    )
                    dirty = True
                out.append(inst)
            if dirty:
                bb.instructions = out


# ---------------------------------------------------------------------------
# Problem constants (hardcoded, self-contained)
# ---------------------------------------------------------------------------
B, S, D = 2, 2048, 1024
H, E = 16, 64           # heads, head dim
NCORES = 8
HL = H // NCORES        # heads per core = 2
BS = B * S              # 4096 flattened rows
ND = D // 128           # 8 d-chunks
ST = 512                # projection s-tile (rhs cols)
NST = BS // ST          # 8
TI = 512                # attention i-tile
NT_I = S // TI          # 4 per batch
TJ = 128                # key chunk
NJC = S // TJ           # 16 per batch
RQ = 256                # rows per A2A slot (half of a TI tile)
ROWS = BS // NCORES     # 512 output rows per core (256 per batch)

f32 = mybir.dt.float32
bf16 = mybir.dt.bfloat16
Exp = mybir.ActivationFunctionType.Exp
Ln = mybir.ActivationFunctionType.Ln
npbf16 = ml_dtypes.bfloat16

_built = [None]


def _build():
    nc = bass.Bass("TRN2", target_bir_lowering=False, debug=False,
                   num_devices=NCORES)

    xt_d = nc.dram_tensor("xt", (D, BS), bf16, kind="ExternalInput").ap()
    wq_d = nc.dram_tensor("wq", (D, 128), bf16, kind="ExternalInput").ap()
    wk_d = nc.dram_tensor("wk", (D, 128), bf16, kind="ExternalInput").ap()
    wv_d = nc.dram_tensor("wv", (D, 128), bf16, kind="ExternalInput").ap()
    bq_d = nc.dram_tensor("bq", (128, 1), f32, kind="ExternalInput").ap()
    bk_d = nc.dram_tensor("bk", (128, 1), f32, kind="ExternalInput").ap()
    bv_d = nc.dram_tensor("bv", (128, 1), f32, kind="ExternalInput").ap()
    wo_d = nc.dram_tensor("wo", (D, D), bf16, kind="ExternalInput").ap()
    bob_d = nc.dram_tensor("bob", (128, D), f32, kind="ExternalInput").ap()
    ident128_d = nc.dram_tensor("ident128", (128, 128), bf16,
                                kind="ExternalInput").ap()
    mask01_d = nc.dram_tensor("mask01", (128, 128), bf16,
                              kind="ExternalInput").ap()
    sel2_d = nc.dram_tensor("sel2", (128, 2 * E), bf16,
                            kind="ExternalInput").ap()

    out_d = nc.dram_tensor("out", (ROWS, D), f32, kind="ExternalOutput").ap()

    # one AllToAll per batch; slot s=2t+half carries [128 feats, 256 rows].
    # b1's A2A is split into two half-row collectives (contiguous tensors)
    # so Wo-b1's first row block starts when half the data has landed.
    a2a_in = [nc.dram_tensor(f"a2a_in{b}", (NCORES, 128, RQ), bf16,
                             kind="Internal").ap() for b in range(B)]
    a2a_out = [nc.dram_tensor(f"a2a_out{b}", (NCORES, 128, RQ), bf16,
                              kind="Internal").ap() for b in range(B)]


    with tile.TileContext(nc) as tc:
        with tc.tile_pool(name="persist", bufs=1) as pp:
            # big activation buffers, feature-on-partition, [2 heads x 64, B*S]
            xt_sb = pp.tile([128, ND, BS], bf16, tag="xt")
            qt = pp.tile([128, BS], bf16, tag="qt")
            kt = pp.tile([128, BS], bf16, tag="kt")
            vt = pp.tile([128, BS], bf16, tag="vt")
            # weights
            wq_sb = pp.tile([128, ND, 128], bf16, tag="wq")
            wk_sb = pp.tile([128, ND, 128], bf16, tag="wk")
            wv_sb = pp.tile([128, ND, 128], bf16, tag="wv")
            wo_sb = pp.tile([128, ND, D], bf16, tag="wo")
            bq_sb = pp.tile([128, 1], f32, tag="bq")
            bk_sb = pp.tile([128, 1], f32, tag="bk")
            bv_sb = pp.tile([128, 1], f32, tag="bv")
            bob_sb = pp.tile([128, D], f32, tag="bob")
            ident128_sb = pp.tile([128, 128], bf16, tag="ident128")
            mask01_sb = pp.tile([128, 128], bf16, tag="mask01")
            sel2_sb = pp.tile([128, 2 * E], bf16, tag="sel2")
            ones1 = pp.tile([128, 1], bf16, tag="ones1")
            # per-(b,t) reciprocal broadcast rows {0,32}; memset once so the
            # unused rows are finite (sel2 zeros them in the matmul)
            recb_all = pp.tile([128, B * NT_I, TI], bf16, tag="recb")
            # A2A gather landing buffers [feat, src core, row]
            g_b = [pp.tile([128, NCORES, RQ], bf16, tag=f"g{b}",
                           name=f"g{b}") for b in range(B)]
            # V natural chunks: per (b, lh): [128 j, NJC, 64]
            vsb = [pp.tile([128, NJC, E], bf16, tag=f"vsb{i}",
                           name=f"vsb{i}")
                   for i in range(B * HL)]

            # weights + x stream interleaved so the first projection can
            # start as soon as wq + x s-tile 0 land; wo is deferred to last
            xt_r = xt_d.rearrange("(c p) s -> p c s", p=128)

            def xslices(st):
                sl = slice(st * ST, (st + 1) * ST)
                nc.sync.dma_start(xt_sb[:, 0:4, sl], xt_r[:, 0:4, sl])
                nc.sync.dma_start(xt_sb[:, 4:8, sl], xt_r[:, 4:8, sl])

            nc.sync.dma_start(wq_sb[:], wq_d.rearrange("(c p) e -> p c e", p=128))
            xslices(0)
            nc.sync.dma_start(wk_sb[:], wk_d.rearrange("(c p) e -> p c e", p=128))
            nc.sync.dma_start(wv_sb[:], wv_d.rearrange("(c p) e -> p c e", p=128))
            nc.sync.dma_start(bq_sb[:], bq_d[:])
            nc.sync.dma_start(bk_sb[:], bk_d[:])
            nc.sync.dma_start(bv_sb[:], bv_d[:])
            xslices(1)
            nc.sync.dma_start(ident128_sb[:], ident128_d[:])
            xslices(2)
            nc.sync.dma_start(mask01_sb[:], mask01_d[:])
            nc.sync.dma_start(sel2_sb[:], sel2_d[:])
            xslices(3)
            for st in range(4, NST):
                xslices(st)
            nc.sync.dma_start(wo_sb[:], wo_d.rearrange("(c p) o -> p c o", p=128))
            nc.sync.dma_start(bob_sb[:], bob_d[:])
            with nc.allow_low_precision(reason="bf16 memset"):
                nc.gpsimd.memset(ones1[:], 1.0)
                nc.gpsimd.memset(recb_all[:], 1.0)

            # ---------------- Phase A: QKV projections + V chunks -----------
            # V^T->V transposes are delayed by one s-tile so the PE never
            # stalls on the DVE copy that materializes vt for that s-tile
            vjobs = []
            with tc.tile_pool(name="ptr", bufs=4, space="PSUM") as ptr_pool, \
                 tc.tile_pool(name="pproj", bufs=3, space="PSUM") as pproj_pool:

                def emit_vjobs(jobs):
                    # one [128,128] transpose flips a key-chunk of BOTH heads:
                    # V^T rows are (h0 dims 0-63 | h1 dims 64-127), so the
                    # transposed block is [128 keys, h0 V | h1 V]
                    for (bb_, jc) in jobs:
                        p_ = ptr_pool.tile([128, 128], bf16, tag="ptr")
                        nc.tensor.transpose(
                            p_[:],
                            vt[:, bb_ * S + jc * TJ: bb_ * S + (jc + 1) * TJ],
                            ident128_sb[:])
                        with nc.allow_low_precision(reason="bf16 V"):
                            for lh in range(HL):
                                nc.vector.tensor_copy(
                                    vsb[bb_ * HL + lh][:, jc, :],
                                    p_[:, lh * E:(lh + 1) * E])

                for st in range(NST):
                    for wsb, bsb, dst in ((wq_sb, bq_sb, qt),
                                          (wk_sb, bk_sb, kt),
                                          (wv_sb, bv_sb, vt)):
                        pp_t = pproj_pool.tile([128, ST], f32, tag="pj")
                        for dc in range(ND):
                            nc.tensor.matmul(
                                pp_t[:], wsb[:, dc, :],
                                xt_sb[:, dc, st * ST:(st + 1) * ST],
                                start=(dc == 0), stop=(dc == ND - 1))
                        with nc.allow_low_precision(reason="bf16 proj"):
                            nc.vector.tensor_scalar_add(
                                dst[:, st * ST:(st + 1) * ST], pp_t[:], bsb[:])
                    emit_vjobs(vjobs)
                    bb_, jc0 = st // 4, 4 * (st % 4)
                    vjobs = [(bb_, jc) for jc in range(jc0, jc0 + 4)]
                emit_vjobs(vjobs)

            # ---------------- Phase B: flash attention, both heads ----------
            # PSUM budget: sc 2x2 + po 2 + pd 2 = 8 banks.  The po pool's
            # full-bank tiles also serve the selector (pb) and Wo (pw).
            with tc.tile_pool(name="sc", bufs=2, space="PSUM") as sc_pool, \
                 tc.tile_pool(name="po", bufs=2, space="PSUM") as po_pool, \
                 tc.tile_pool(name="pd", bufs=2, space="PSUM") as pd_pool, \
                 tc.tile_pool(name="es", bufs=4) as es_pool, \
                 tc.tile_pool(name="osb", bufs=2) as osbp, \
                 tc.tile_pool(name="ost", bufs=8) as ostp, \
                 tc.tile_pool(name="rec", bufs=4) as recp, \
                 tc.tile_pool(name="ob", bufs=4) as ob_pool:

                def emit_pvd(jobs, po, pdn, t):
                    # P@V col-tiled pair per chunk (h0 -> psum partitions
                    # 0:64, h1 -> 64:128; M=64 each, concurrent), then the
                    # denominator quad: 4 M=1 ones-matmuls at col groups
                    # 0..3 (2 heads x 2 chunks) -> rows {0,32,64,96} of pdn
                    last = 4 * t + 3
                    for (b, jc, ncols, coff, es, ci) in jobs:
                        for h in range(HL):
                            nc.tensor.matmul(
                                po[E * h:E * (h + 1), coff:TI],
                                vsb[b * HL + h][:, jc, :],
                                es[:, h, 0:ncols],
                                start=(jc == 0), stop=(jc == last))
                    for (b, jc, ncols, coff, es, ci) in jobs:
                        p = ci % 2
                        for h in range(HL):
                            r = 32 * (2 * h + p)
                            nc.tensor.matmul(
                                pdn[r:r + 1, coff:TI],
                                ones1[:],
                                es[:, h, 0:ncols],
                                start=(ci < 2), stop=(ci >= last - 1),
                                tile_position=(0, r))

                def emit_epi_front(b, t, po, pdn):
                    # d = d_even + d_odd (DVE), 1/d = exp(-ln d) on the
                    # scalar engine -> recg_big rows {0,32}.  For t=0 the
                    # odd-parity rows never received columns 0:128 (no odd
                    # chunk covers them), so copy the even row there.
                    recg_big = recb_all[:, 4 * b + t, :]
                    for h in range(HL):
                        # DVE tensor_tensor rejects mismatched partition
                        # bases; copies don't -- stage both rows at base 0
                        cp0 = recp.tile([1, TI], f32, tag="c0",
                                        name=f"c0_{b}_{t}_{h}")
                        cp1 = recp.tile([1, TI], f32, tag="c1",
                                        name=f"c1_{b}_{t}_{h}")
                        nc.vector.tensor_copy(cp0[:],
                                              pdn[64 * h:64 * h + 1, :])
                        c0 = 128 if t == 0 else 0
                        if c0:
                            nc.vector.memset(cp1[:, 0:c0], 0.0)
                        nc.vector.tensor_copy(
                            cp1[:, c0:TI],
                            pdn[64 * h + 32:64 * h + 33, c0:TI])
                        dsum = recp.tile([1, TI], f32, tag="ds",
                                         name=f"ds{b}_{t}_{h}")
                        nc.vector.tensor_add(dsum[:], cp0[:], cp1[:])
                        lnd = recp.tile([1, TI], f32, tag="ln",
                                        name=f"ln{b}_{t}_{h}")
                        nc.scalar.activation(lnd[:], dsum[:], Ln)
                        with nc.allow_low_precision(reason="denom bf16"):
                            nc.scalar.activation(
                                recg_big[32 * h:32 * h + 1, :], lnd[:],
                                Exp, scale=-1.0)
                    # O copy-out AFTER the denominator chain: the deferred
                    # selector matmuls wait only on recg_big, so the 690ns
                    # CAST must not sit ahead of the cp/dsum copies in the
                    # DVE queue
                    osb = osbp.tile([128, TI], bf16, tag="ob",
                                    name=f"osb{b}_{t}")
                    with nc.allow_low_precision(reason="bf16 O"):
                        nc.vector.tensor_copy(osb[:], po[:])
                    return osb

                def emit_epi_back(b, t, osb):
                    # deferred into the NEXT tile's stream so the selector
                    # matmuls never stall the PE on the scalar-engine chain
                    recg_big = recb_all[:, 4 * b + t, :]
                    pb = po_pool.tile([128, TI], f32, tag="o",
                                      name=f"pb{b}_{t}")
                    for h in range(HL):
                        nc.tensor.matmul(
                            pb[E * h:E * (h + 1), :],
                            sel2_sb[:, h * E:(h + 1) * E],
                            recg_big[:], start=True, stop=True)
                    for h in range(HL):
                        ost = ostp.tile([E, TI], bf16, tag="ost")
                        with nc.allow_low_precision(reason="bf16 ost"):
                            nc.vector.tensor_mul(
                                ost[:], osb[E * h:E * (h + 1), :],
                                pb[E * h:E * (h + 1), :])
                        for half in range(2):
                            nc.sync.dma_start(
                                a2a_in[b][2 * t + half,
                                          E * h:E * (h + 1), :],
                                ost[:, RQ * half:RQ * (half + 1)])

                pending = [None]

                def flush_pending():
                    if pending[0] is not None:
                        emit_epi_back(*pending[0])
                        pending[0] = None

                def emit_collective(b):
                    nc.gpsimd.collective_compute(
                        "AllToAll", mybir.AluOpType.bypass,
                        replica_groups=[list(range(NCORES))],
                        ins=[a2a_in[b][:]], outs=[a2a_out[b][:]])

                for b in range(B):
                    for t in range(NT_I):
                        po = po_pool.tile([128, TI], f32, tag="o",
                                          name=f"po{b}_{t}")
                        pdn = pd_pool.tile([128, TI], f32, tag="pd",
                                           name=f"pd{b}_{t}")
                        # chunk list: full chunks then column-shrunk diagonal
                        chunks = [(jc, TI, 0) for jc in range(4 * t)]
                        chunks += [(4 * t + ri, TI - 128 * ri, 128 * ri)
                                   for ri in range(4)]
                        pvd_jobs = []
                        prev_jobs = None
                        for ci, (jc, ncols, coff) in enumerate(chunks):
                            scp = sc_pool.tile([128, HL, TI], f32, tag="s")
                            for h in range(HL):
                                nc.tensor.matmul(
                                    scp[:, h, 0:ncols],
                                    kt[E * h:E * (h + 1),
                                       b * S + jc * TJ: b * S + (jc + 1) * TJ],
                                    qt[E * h:E * (h + 1),
                                       b * S + t * TI + coff:
                                       b * S + t * TI + coff + ncols],
                                    start=True, stop=True)
                            es = es_pool.tile([128, HL, TI], bf16, tag="e")
                            with nc.allow_low_precision(reason="bf16 exp"):
                                if ncols == TI:
                                    # both heads in one [128,1024] exp
                                    nc.scalar.activation(es[:, :, :],
                                                         scp[:, :, :], Exp,
                                                         scale=0.125)
                                else:
                                    for h in range(HL):
                                        nc.scalar.activation(
                                            es[:, h, 0:ncols],
                                            scp[:, h, 0:ncols], Exp,
                                            scale=0.125)
                                if coff or jc == 4 * t:
                                    # zero the causally-invalid upper
                                    # triangle of the leading 128 cols
                                    for h in range(HL):
                                        nc.vector.tensor_mul(es[:, h, 0:128],
                                                             es[:, h, 0:128],
                                                             mask01_sb[:])
                            pvd_jobs.append((b, jc, ncols, coff, es, ci))
                            if len(pvd_jobs) == 2:
                                if prev_jobs is not None:
                                    emit_pvd(prev_jobs, po, pdn, t)
                                prev_jobs = pvd_jobs
                                pvd_jobs = []
                            if ci == 3:
                                flush_pending()
                                if b == 1 and t == 0:
                                    # b0's slots are complete; launch its
                                    # A2A under b1's attention and stage
                                    # the results into SBUF as they land
                                    emit_collective(0)
                                    nc.sync.dma_start(
                                        g_b[0][:],
                                        a2a_out[0]
                                        .rearrange("f e r -> e f r"))
                        if prev_jobs is not None:
                            emit_pvd(prev_jobs, po, pdn, t)
                        osb = emit_epi_front(b, t, po, pdn)
                        pending[0] = (b, t, osb)

                # ------- Phase C: Wo.  b0's groups overlap b1's AllToAll;
                # only b1's half is a serial tail.  The b1 gather is split
                # per row-block and emitted after b0's Wo so b0's output
                # DMAs don't queue behind its wait on the collective.
                def wo_group(b, rb, ot):
                    pw = po_pool.tile([128, TI], f32, tag="o",
                                      name=f"pw{b}_{rb}_{ot}")
                    for fi in range(NCORES):
                        nc.tensor.matmul(
                            pw[:],
                            g_b[b][:, fi, rb * 128:(rb + 1) * 128],
                            wo_sb[:, fi, ot * 512:(ot + 1) * 512],
                            start=(fi == 0), stop=(fi == NCORES - 1))
                    ob = ob_pool.tile([128, 512], f32, tag="obo")
                    nc.vector.tensor_add(
                        ob[:], pw[:], bob_sb[:, ot * 512:(ot + 1) * 512])
                    nc.sync.dma_start(
                        out_d[b * RQ + rb * 128: b * RQ + (rb + 1) * 128,
                              ot * 512:(ot + 1) * 512],
                        ob[:])

                wo_group(0, 0, 0)
                flush_pending()
                emit_collective(1)
                wo_group(0, 0, 1)
                wo_group(0, 1, 0)
                wo_group(0, 1, 1)
                g1r = a2a_out[1].rearrange("f e r -> e f r")
                for rb in range(2):
                    nc.sync.dma_start(
                        g_b[1][:, :, 128 * rb:128 * (rb + 1)],
                        g1r[:, :, 128 * rb:128 * (rb + 1)])
                    for ot in range(2):
                        wo_group(1, rb, ot)

    _split_multi_waits(nc)
    return nc


def _get_nc():
    if _built[0] is None:
        _built[0] = _build()
    return _built[0]


def _host_inputs(x, Wq, bq, Wk, bk, Wv, bv, Wo, bo):
    xT = np.ascontiguousarray(
        np.asarray(x, dtype=np.float32).reshape(BS, D).T).astype(npbf16)
    Wq = np.asarray(Wq, dtype=np.float32)
    Wk = np.asarray(Wk, dtype=np.float32)
    Wv = np.asarray(Wv, dtype=np.float32)
    bq = np.asarray(bq, dtype=np.float32)
    bk = np.asarray(bk, dtype=np.float32)
    bv = np.asarray(bv, dtype=np.float32)
    Wo = np.ascontiguousarray(np.asarray(Wo, dtype=np.float32)).astype(npbf16)
    bo = np.asarray(bo, dtype=np.float32)

    ident128 = np.eye(128).astype(npbf16)
    jj = np.arange(128, dtype=np.int64)[:, None]
    ii = np.arange(128, dtype=np.int64)[None, :]
    mask01 = (jj <= ii).astype(npbf16)
    sel2 = np.zeros((128, 2 * E), dtype=npbf16)
    for h in range(2):
        sel2[32 * h, h * E:(h + 1) * E] = 1.0
    bob = np.tile(bo[None, :], (128, 1)).astype(np.float32)

    in_maps = []
    for c in range(NCORES):
        hs = slice(HL * c, HL * (c + 1))
        in_maps.append({
            "xt": xT,
            "wq": np.ascontiguousarray(
                Wq[hs].transpose(1, 0, 2).reshape(D, 128)).astype(npbf16),
            "wk": np.ascontiguousarray(
                Wk[hs].transpose(1, 0, 2).reshape(D, 128)).astype(npbf16),
            "wv": np.ascontiguousarray(
                Wv[hs].transpose(1, 0, 2).reshape(D, 128)).astype(npbf16),
            "bq": np.ascontiguousarray(bq[hs].reshape(128, 1)),
            "bk": np.ascontiguousarray(bk[hs].reshape(128, 1)),
            "bv": np.ascontiguousarray(bv[hs].reshape(128, 1)),
            "wo": Wo,
            "bob": bob,
            "ident128": ident128,
            "mask01": mask01,
            "sel2": sel2,
        })
    return in_maps


def kernel(x, Wq, bq, Wk, bk, Wv, bv, Wo, bo, _trace=False, _tmpdir=None):
    nc = _get_nc()
    in_maps = _host_inputs(x, Wq, bq, Wk, bk, Wv, bv, Wo, bo)
    res = bass_utils.run_bass_kernel_spmd(
        nc, in_maps, core_ids=list(range(NCORES)),
        trace=_trace, tmpdir=_tmpdir)
    # core c returns [512, 1024]: rows 0:256 = batch0, 256:512 = batch1 of
    # global row block 512*(c//2) + 256*(c%2)
    out = np.empty((B, S, D), dtype=np.float32)
    for c in range(NCORES):
        gr = 512 * (c // 2) + 256 * (c % 2)
        blk = res.results[c]["out"]
        for b in range(B):
            out[b, gr:gr + RQ, :] = blk[b * RQ:(b + 1) * RQ, :]
    kernel.last_exec_time_ns = res.exec_time_ns
    kernel.last_results = res
    return out


kernel.last_exec_time_ns = None
kernel.last_results = None
